# revision 1
# baseline (speedup 1.0000x reference)
"""CoherenceAttention Trainium2 kernel.

Strategy (see spec sharding_hint): data-parallel over batch — B=8 batch rows,
one NeuronCore each, pure SPMD, no collectives. Host does index preprocessing
(membership masks from sentence_boundaries) and parameter folding; the device
does all O(B*T*H*H) compute.

Math rewrite vs the reference:
  - sent_repr = member_pool.T @ h      (member_pool = member / counts, [T,S])
  - qk = Wqk sent_repr + bqk           (q pre-scaled by 1/sqrt(DH) on host)
  - v  = sent_repr Wv.T + bv
  - per-head: scores = qT.T kT; softmax (no max-sub needed: tiny logits);
    ctxT = v_h @ attn.T
  - attended @ W1b.T  ==  ctx @ (W1b @ out_w).T + (W1b @ out_b + b1)
    (merged on host into Wm / biasm; b1 folded here is only valid for covered
     tokens, which is fine because uncovered tokens' scores get masked to 0)
  - z.T = W1a.T.T @ h.T + z2'.T-gather  (gather = memberF.T as matmul operand)
  - score = w2 . relu(z);  out = h * (1 + covered * (score + b2))
"""

import numpy as np
import ml_dtypes

import concourse.bass as bass
import concourse.tile as tile
from concourse import mybir
from concourse.bass_utils import run_bass_kernel_spmd
from concourse.masks import make_identity

B, T, H, S, NH = 8, 4096, 1024, 64, 16
DH = H // NH
P = 128
TT = T // P          # 32 token tiles
KD = H // P          # 8 contraction tiles over H
NJ = H // P          # 8 j-tiles (scorer hidden dim)
TC = 8               # token chunks in the scorer loop
CW = T // TC         # 512 tokens per chunk
NMT = 2 * H // P     # 16 m-tiles for fused q|k projection

F32 = mybir.dt.float32
BF16 = mybir.dt.bfloat16
BF = ml_dtypes.bfloat16

_CACHE = {}


def _split_multi_waits(nc: bass.Bass) -> None:
    """The pinned walrus rejects >1 sync-wait per instruction ("Too many sync
    wait commands"). Hoist extra waits onto same-engine NoOps placed right
    before the instruction — semantically equivalent (sequential waits)."""
    uid = 0
    for fn in nc.m.functions:
        for blk in fn.blocks:
            out = []
            changed = False
            for inst in blk.instructions:
                si = inst.sync_info
                if si is not None and len(si.on_wait) > 1:
                    waits = list(si.on_wait)
                    for w in waits[:-1]:
                        nop = mybir.InstNoOp(
                            name=f"waitsplit-{uid}",
                            engine=inst.engine,
                            sync_info=mybir.SyncInfo(on_wait=[w], on_update=[]),
                        )
                        uid += 1
                        nc.register_instruction(nop, overwrite=True)
                        out.append(nop)
                    si.on_wait = [waits[-1]]
                    inst.sync_info = si
                    changed = True
                out.append(inst)
            if changed:
                blk.instructions = out


def _build(b2val: float) -> bass.Bass:
    nc = bass.Bass("TRN2", target_bir_lowering=False, debug=False, num_devices=B)

    h_d = nc.dram_tensor("h", (T, H), F32, kind="ExternalInput")
    hbf_d = nc.dram_tensor("hbf", (T, H), BF16, kind="ExternalInput")
    hT_d = nc.dram_tensor("ht", (H, T), BF16, kind="ExternalInput")
    mpool_d = nc.dram_tensor("mpool", (T, S), BF16, kind="ExternalInput")
    mfT_d = nc.dram_tensor("mft", (S, T), BF16, kind="ExternalInput")
    covT_d = nc.dram_tensor("covt", (P, TT), F32, kind="ExternalInput")
    wqkT_d = nc.dram_tensor("wqkt", (NMT, P, H), BF16, kind="ExternalInput")
    bqk_d = nc.dram_tensor("bqk", (P, NMT), F32, kind="ExternalInput")
    wvT_d = nc.dram_tensor("wvt", (H, H), BF16, kind="ExternalInput")
    bv_d = nc.dram_tensor("bv", (1, H), BF16, kind="ExternalInput")
    w1aT_d = nc.dram_tensor("w1at", (H, H), BF16, kind="ExternalInput")
    wmT_d = nc.dram_tensor("wmt", (H, H), BF16, kind="ExternalInput")
    biasm_d = nc.dram_tensor("biasm", (1, H), BF16, kind="ExternalInput")
    w2T_d = nc.dram_tensor("w2t", (P, NJ), BF16, kind="ExternalInput")
    out_d = nc.dram_tensor("out", (T, H), F32, kind="ExternalOutput")

    PF = 2  # hT chunk prefetch depth

    with tile.TileContext(nc) as tc:
        with tc.tile_pool(name="singles", bufs=1) as singles:
            # resident across the whole kernel (~32 KiB/partition)
            w1aT_sb = singles.tile([P, KD, H], BF16)      # 16 KiB/part
            mfT_sb = singles.tile([S, T], BF16)           # 8 KiB/part
            covT_sb = singles.tile([P, TT], F32)
            bqk_sb = singles.tile([P, NMT], F32)
            w2T_sb = singles.tile([P, NJ], BF16)
            sentT_bf = singles.tile([P, KD, S], BF16)
            qkT_bf = singles.tile([P, NMT, S], BF16)
            v_bf = singles.tile([S, H], BF16)
            ctxT_bf = singles.tile([P, KD, S], BF16)
            z2_bf = singles.tile([S, H], BF16)

            nc.sync.dma_start(covT_sb, covT_d.ap())
            nc.sync.dma_start(bqk_sb, bqk_d.ap())
            nc.sync.dma_start(w2T_sb, w2T_d.ap())

            # ---------- phase A: segment mean pooling + sent transpose ----------
            # sent_repr[s, d] = sum_t member_pool[t, s] * h[t, d]  (bf16 inputs)
            with tc.tile_pool(name="pha", bufs=1) as pha:
                with tc.tile_pool(name="psA", bufs=1, space="PSUM") as psA:
                    mp_sb = pha.tile([P, TT, S], BF16)
                    sent_sb = pha.tile([S, H], F32)
                    ident32 = pha.tile([P, P], F32)
                    make_identity(nc, ident32)
                    for i in range(TT):
                        nc.sync.dma_start(
                            mp_sb[:, i, :], mpool_d[i * P:(i + 1) * P, :]
                        )
                    ps0 = psA.tile([S, 512], F32, tag="pool0", bufs=1)
                    ps1 = psA.tile([S, 512], F32, tag="pool1", bufs=1)
                    for kt in range(TT):
                        hbf = pha.tile([P, H], BF16, tag="hbf", bufs=8)
                        nc.sync.dma_start(hbf, hbf_d[kt * P:(kt + 1) * P, :])
                        nc.tensor.matmul(
                            ps0, mp_sb[:, kt, :], hbf[:, 0:512],
                            start=(kt == 0), stop=(kt == TT - 1),
                        )
                        nc.tensor.matmul(
                            ps1, mp_sb[:, kt, :], hbf[:, 512:1024],
                            start=(kt == 0), stop=(kt == TT - 1),
                        )
                    nc.vector.tensor_copy(out=sent_sb[:, 0:512], in_=ps0)
                    nc.vector.tensor_copy(out=sent_sb[:, 512:1024], in_=ps1)
                    # transpose sent_repr -> sentT (d on partitions), cast bf16
                    for k in range(KD):
                        pst = psA.tile([P, S], F32, tag="tr", bufs=2)
                        nc.tensor.transpose(
                            pst, sent_sb[:, k * P:(k + 1) * P], ident32[:S, :S]
                        )
                        nc.vector.tensor_copy(out=sentT_bf[:, k, :], in_=pst)

            # open phase-C SBUF pool early so hT chunks + w1a/mfT prefetch
            # ahead of the attention phase's weight streams
            with tc.tile_pool(name="phc", bufs=2) as phc:
                htc_tiles = {}

                def prefetch_htc(c):
                    htc = phc.tile([P, KD, CW], BF16, tag="ht", bufs=PF + 1)
                    for k in range(KD):
                        nc.sync.dma_start(
                            htc[:, k, :],
                            hT_d[k * P:(k + 1) * P, c * CW:(c + 1) * CW],
                        )
                    htc_tiles[c] = htc

                # ---------- phase B: q|k, v, attention, merged out-proj ----------
                with tc.tile_pool(name="phb", bufs=4) as phb:
                    identbf = phb.tile([P, P], BF16, bufs=1)
                    ones_bf = phb.tile([1, S], BF16, bufs=1)
                    bv_sb = phb.tile([1, H], BF16, bufs=1)
                    biasm_sb = phb.tile([1, H], BF16, bufs=1)
                    make_identity(nc, identbf)
                    nc.vector.memset(ones_bf, 1.0)
                    nc.sync.dma_start(bv_sb, bv_d.ap())
                    nc.sync.dma_start(biasm_sb, biasm_d.ap())

                    with tc.tile_pool(name="psB1", bufs=1, space="PSUM") as psB1:
                        # q|k rows: qkT[m, s] = sum_d wqkT[d, m] sentT[d, s] + bqk
                        for mt in range(NMT):
                            wt = phb.tile([P, KD, P], BF16, tag="wqk", bufs=6)
                            nc.sync.dma_start(
                                wt, wqkT_d[mt].rearrange("p (k m) -> p k m", k=KD)
                            )
                            psqk = psB1.tile([P, S], F32, tag="qk", bufs=2)
                            for k in range(KD):
                                nc.tensor.matmul(
                                    psqk, wt[:, k, :], sentT_bf[:, k, :],
                                    start=(k == 0), stop=(k == KD - 1),
                                )
                            nc.vector.tensor_scalar(
                                out=qkT_bf[:, mt, :], in0=psqk,
                                scalar1=bqk_sb[:, mt:mt + 1], scalar2=None,
                                op0=mybir.AluOpType.add,
                            )

                        # v[s, d'] = sum_d sentT[d, s] wvT[d, d'] + bv
                        psv = psB1.tile([S, H], F32, tag="v", bufs=1)
                        for k in range(KD):
                            wv = phb.tile([P, H], BF16, tag="wv", bufs=5)
                            nc.sync.dma_start(wv, wvT_d[k * P:(k + 1) * P, :])
                            for nb in range(2):
                                nc.tensor.matmul(
                                    psv[:, nb * 512:(nb + 1) * 512],
                                    sentT_bf[:, k, :],
                                    wv[:, nb * 512:(nb + 1) * 512],
                                    start=(k == 0), stop=False,
                                )
                        for nb in range(2):
                            nc.tensor.matmul(
                                psv[:, nb * 512:(nb + 1) * 512],
                                ones_bf, bv_sb[:1, nb * 512:(nb + 1) * 512],
                                start=False, stop=True,
                            )
                        nc.vector.tensor_copy(out=v_bf, in_=psv)

                    # queue phase-C feeds behind the q|k/v weight streams but
                    # ahead of wm, so the scorer can start right after z2'
                    for k in range(KD):
                        nc.sync.dma_start(
                            w1aT_sb[:, k, :], w1aT_d[k * P:(k + 1) * P, :]
                        )
                    nc.sync.dma_start(mfT_sb, mfT_d.ap())
                    for c in range(PF):
                        prefetch_htc(c)

                    with tc.tile_pool(name="psB2", bufs=1, space="PSUM") as psB2:
                        # attention heads (per-head softmax, v1 style)
                        for hh in range(NH):
                            po = (hh % 2) * 64
                            mt = hh // 2
                            qT_h = qkT_bf[po:po + 64, mt, :]
                            kT_h = qkT_bf[po:po + 64, NJ + mt, :]
                            ps_sc = psB2.tile([S, S], F32, tag="sc", bufs=2)
                            nc.tensor.matmul(ps_sc, qT_h, kT_h, start=True, stop=True)
                            ex = phb.tile([S, S], F32, tag="ex")
                            nc.scalar.activation(
                                out=ex, in_=ps_sc, func=mybir.ActivationFunctionType.Exp
                            )
                            sm = phb.tile([S, 1], F32, tag="sm")
                            nc.vector.reduce_sum(out=sm, in_=ex, axis=mybir.AxisListType.X)
                            nc.vector.reciprocal(out=sm, in_=sm)
                            at = phb.tile([S, S], BF16, tag="at")
                            nc.vector.tensor_scalar_mul(out=at, in0=ex, scalar1=sm)
                            ps_t = psB2.tile([S, S], BF16, tag="att", bufs=2)
                            nc.tensor.transpose(ps_t, at, identbf[:S, :S])
                            atT = phb.tile([S, S], BF16, tag="atT")
                            nc.vector.tensor_copy(out=atT, in_=ps_t)
                            ps_c = psB2.tile([S, S], F32, tag="ctx", bufs=2)
                            nc.tensor.matmul(
                                ps_c, v_bf[:, hh * 64:(hh + 1) * 64], atT,
                                start=True, stop=True,
                            )
                            nc.vector.tensor_copy(out=ctxT_bf[po:po + 64, mt, :], in_=ps_c)

                    with tc.tile_pool(name="psB3", bufs=1, space="PSUM") as psB3:
                        # z2'[s, j] = sum_d' ctxT[d', s] wmT[d', j] + biasm
                        psz2 = psB3.tile([S, H], F32, tag="z2", bufs=1)
                        for k in range(KD):
                            wm = phb.tile([P, H], BF16, tag="wm", bufs=5)
                            nc.sync.dma_start(wm, wmT_d[k * P:(k + 1) * P, :])
                            for nb in range(2):
                                nc.tensor.matmul(
                                    psz2[:, nb * 512:(nb + 1) * 512],
                                    ctxT_bf[:, k, :],
                                    wm[:, nb * 512:(nb + 1) * 512],
                                    start=(k == 0), stop=False,
                                )
                        for nb in range(2):
                            nc.tensor.matmul(
                                psz2[:, nb * 512:(nb + 1) * 512],
                                ones_bf, biasm_sb[:1, nb * 512:(nb + 1) * 512],
                                start=False, stop=True,
                            )
                        nc.vector.tensor_copy(out=z2_bf, in_=psz2)

                # ---------- phase C: scorer over token chunks ----------
                with tc.tile_pool(name="psC", bufs=1, space="PSUM") as psC:
                    one32 = phc.tile([1, 1], F32, bufs=1)
                    nc.vector.memset(one32, 1.0)
                    nt = CW // P
                    for c in range(TC):
                        if c + PF < TC:
                            prefetch_htc(c + PF)
                        htc = htc_tiles.pop(c)
                        hs = phc.tile([P, nt, H], F32, tag="hs", bufs=2)
                        for a in range(nt):
                            it = nt * c + a
                            nc.sync.dma_start(
                                hs[:, a, :], h_d[it * P:(it + 1) * P, :]
                            )
                        ps_s = psC.tile([1, CW], F32, tag="score", bufs=2)
                        hids = []
                        for jt in range(NJ):
                            ps_z = psC.tile([P, CW], F32, tag="z", bufs=4)
                            for k in range(KD):
                                nc.tensor.matmul(
                                    ps_z, w1aT_sb[:, k, jt * P:(jt + 1) * P],
                                    htc[:, k, :],
                                    start=(k == 0), stop=False,
                                )
                            nc.tensor.matmul(
                                ps_z, z2_bf[:, jt * P:(jt + 1) * P],
                                mfT_sb[:, c * CW:(c + 1) * CW],
                                start=False, stop=True,
                            )
                            hid = phc.tile([P, CW], BF16, tag="hid", bufs=3)
                            nc.scalar.activation(
                                out=hid, in_=ps_z,
                                func=mybir.ActivationFunctionType.Relu,
                            )
                            hids.append(hid)
                            # skew the score matmul one j-tile behind the relu
                            if jt > 0:
                                nc.tensor.matmul(
                                    ps_s, w2T_sb[:, jt - 1:jt], hids[jt - 1],
                                    start=(jt == 1), stop=False,
                                )
                        nc.tensor.matmul(
                            ps_s, w2T_sb[:, NJ - 1:NJ], hids[NJ - 1],
                            start=False, stop=True,
                        )
                        # sc1 = score + b2; transpose [1,128]->[128,1] via PE
                        sc1 = phc.tile([1, CW], F32, tag="sc1", bufs=2)
                        nc.vector.tensor_scalar_add(
                            out=sc1, in0=ps_s, scalar1=float(b2val)
                        )
                        psT = psC.tile([P, nt], F32, tag="scT", bufs=2)
                        for a in range(nt):
                            nc.tensor.matmul(
                                psT[:, a:a + 1], sc1[0:1, a * P:(a + 1) * P],
                                one32, start=True, stop=True,
                            )
                        # scale = 1 + covered * (score + b2); out = h * scale
                        scf = phc.tile([P, nt], F32, tag="scf", bufs=2)
                        nc.vector.tensor_mul(
                            out=scf, in0=psT, in1=covT_sb[:, c * nt:(c + 1) * nt]
                        )
                        nc.vector.tensor_scalar_add(out=scf, in0=scf, scalar1=1.0)
                        for a in range(nt):
                            it = nt * c + a
                            nc.vector.tensor_scalar_mul(
                                out=hs[:, a, :], in0=hs[:, a, :],
                                scalar1=scf[:, a:a + 1],
                            )
                            nc.sync.dma_start(
                                out_d[it * P:(it + 1) * P, :], hs[:, a, :]
                            )
    _split_multi_waits(nc)
    return nc


def _preprocess(context_hidden, sentence_boundaries, in_proj_w, in_proj_b,
                out_w, out_b, w1, b1, w2, b2):
    """Host-side index preprocessing + parameter folding (shared across cores)."""
    starts = np.asarray(sentence_boundaries)[:, :, 0].astype(np.int64)   # [B,S]
    ends = np.asarray(sentence_boundaries)[:, :, 1].astype(np.int64)     # [B,S]
    t = np.arange(T, dtype=np.int64)
    member = (t[None, :, None] >= starts[:, None, :]) & (
        t[None, :, None] < ends[:, None, :]
    )                                                        # [B,T,S]
    mf = member.astype(np.float32)
    counts = np.clip(mf.sum(axis=1), 1.0, None)              # [B,S]
    mpool = mf / counts[:, None, :]                          # [B,T,S]
    sid = np.argmax(member, axis=2)                          # [B,T] first True
    covered = member.any(axis=2)                             # [B,T]
    memberF = np.eye(S, dtype=np.float32)[sid] * covered[..., None].astype(np.float32)
    mfT = np.ascontiguousarray(memberF.transpose(0, 2, 1)).astype(BF)  # [B,S,T]
    # covT[p, i] = covered[i*128 + p]
    covT = np.ascontiguousarray(
        covered.astype(np.float32).reshape(B, TT, P).transpose(0, 2, 1)
    )                                                        # [B,128,32]

    scale = 1.0 / np.sqrt(np.float32(DH))
    wqk = np.asarray(in_proj_w)[:2 * H, :].astype(np.float32).copy()     # [2H, H]
    wqk[:H] *= scale
    bqk = np.asarray(in_proj_b)[:2 * H].astype(np.float32).copy()
    bqk[:H] *= scale
    # tiled stationary layout: wqkt[mt, p, k*128+m2] = wqkT[k*128+p, mt*128+m2]
    wqk_t = np.ascontiguousarray(
        wqk.T.reshape(KD, P, NMT, P).transpose(2, 1, 0, 3).reshape(NMT, P, H)
    ).astype(BF)
    bqk_t = np.ascontiguousarray(bqk.reshape(NMT, P).T)      # [128, 16] f32

    wvT = np.ascontiguousarray(
        np.asarray(in_proj_w)[2 * H:, :].astype(np.float32).T
    ).astype(BF)
    bv_row = np.asarray(in_proj_b)[2 * H:].astype(np.float32)[None, :].astype(BF)

    w1_np = np.asarray(w1).astype(np.float32)
    w1aT = np.ascontiguousarray(w1_np[:, :H].T).astype(BF)
    W1b = w1_np[:, H:]                                       # [H, H]
    Wm = W1b @ np.asarray(out_w).astype(np.float32)          # [j, d']
    wmT = np.ascontiguousarray(Wm.T).astype(BF)
    biasm = (
        W1b @ np.asarray(out_b).astype(np.float32)
        + np.asarray(b1).astype(np.float32)
    )[None, :].astype(BF)
    w2t = np.ascontiguousarray(
        np.asarray(w2)[0].astype(np.float32).reshape(NJ, P).T
    ).astype(BF)
    b2val = float(np.asarray(b2).reshape(-1)[0])

    shared = dict(
        wqkt=wqk_t, bqk=bqk_t, wvt=wvT, bv=bv_row, w1at=w1aT,
        wmt=wmT, biasm=biasm, w2t=w2t,
    )
    in_maps = []
    for b in range(B):
        hb = np.ascontiguousarray(np.asarray(context_hidden)[b]).astype(np.float32)
        in_maps.append(dict(
            shared,
            h=hb,
            ht=np.ascontiguousarray(hb.T).astype(BF),
            hbf=hb.astype(BF),
            mpool=np.ascontiguousarray(mpool[b]).astype(BF),
            mft=np.ascontiguousarray(mfT[b]),
            covt=np.ascontiguousarray(covT[b]),
        ))
    return in_maps, b2val


def kernel(**inputs) -> np.ndarray:
    in_maps, b2val = _preprocess(**inputs)
    key = ("nc", b2val)
    if key not in _CACHE:
        _CACHE[key] = _build(b2val)
    nc = _CACHE[key]
    res = run_bass_kernel_spmd(nc, in_maps, core_ids=list(range(B)))
    out = np.stack([res.results[b]["out"] for b in range(B)], axis=0)
    return out.astype(np.float32)



# revision 6
# speedup vs baseline: 1.5219x; 1.5219x over previous
"""CoherenceAttention Trainium2 kernel.

Strategy (see spec sharding_hint): data-parallel over batch — B=8 batch rows,
one NeuronCore each, pure SPMD, no collectives. Host does index preprocessing
(membership masks from sentence_boundaries) and parameter folding; the device
does all O(B*T*H*H) compute.

v2 math/layout rewrite:
  - z in [t, j] layout: psZ[t-tile 128, j 1024] = z1 + gather + bias.
  - z1 = h @ W1a.T at fp8 with a 3-term hi/lo residual split (DoubleRow perf
    mode, 0.5 cyc/row): h ~ hi + lo/RK, W ~ whi + wlo/RK =>
    z1 ~ hi@whi + lo@(whi/RK) + hi@(wlo/RK); bf16-grade accuracy at 0.75x
    bf16 PE cost.
  - sentence pooling, q|k, v, z2' projections all fp8 DoubleRow.
  - gather of per-sentence z2' to tokens as an fp8 DoubleRow matmul with the
    s=64 contraction packed [32, 2].
  - score = w2 . relu(z) fused into ONE DVE op per tile:
    scalar_tensor_tensor(max(psZ,0) * w2rep, accum_out=score).
  - out = h*(1+cov*(score+b2)) as one more DVE op from resident bf16 h;
    output written bf16 and upcast on host (within tolerance).
"""

import numpy as np
import ml_dtypes

import concourse.bass as bass
import concourse.tile as tile
from concourse import mybir
from concourse.bass_utils import run_bass_kernel_spmd
from concourse.masks import make_identity

B, T, H, S, NH = 8, 4096, 1024, 64, 16
DH = H // NH
P = 128
TT = T // P          # 32 token tiles
KD = H // P          # 8 contraction tiles over H
KP = KD // 2         # 4 DoubleRow pair-chunks
NMT = 2 * H // P     # 16 m-tiles for fused q|k projection

AW = 64.0            # w1a / z scale for fp8
RK = 16.0            # residual split scale
AQK = 16.0           # q/k weight scale
AV = 16.0            # v weight scale
AM = 64.0            # Wm weight scale

F32 = mybir.dt.float32
BF16 = mybir.dt.bfloat16
F8 = mybir.dt.float8e4
BF = ml_dtypes.bfloat16
F8NP = ml_dtypes.float8_e4m3
DR = mybir.MatmulPerfMode.DoubleRow

_CACHE = {}


def _split_multi_waits(nc: bass.Bass) -> None:
    """The pinned walrus rejects >1 sync-wait per instruction ("Too many sync
    wait commands"). Hoist extra waits onto same-engine NoOps placed right
    before the instruction — semantically equivalent (sequential waits)."""
    uid = 0
    for fn in nc.m.functions:
        for blk in fn.blocks:
            out = []
            changed = False
            for inst in blk.instructions:
                si = inst.sync_info
                if si is not None and len(si.on_wait) > 1:
                    waits = list(si.on_wait)
                    for w in waits[:-1]:
                        nop = mybir.InstNoOp(
                            name=f"waitsplit-{uid}",
                            engine=inst.engine,
                            sync_info=mybir.SyncInfo(on_wait=[w], on_update=[]),
                        )
                        uid += 1
                        nc.register_instruction(nop, overwrite=True)
                        out.append(nop)
                    si.on_wait = [waits[-1]]
                    inst.sync_info = si
                    changed = True
                out.append(inst)
            if changed:
                blk.instructions = out


def _build(b2val: float) -> bass.Bass:
    nc = bass.Bass("TRN2", target_bir_lowering=False, debug=False, num_devices=B)

    mp8_d = nc.dram_tensor("mp8", (P, TT * S), F8, kind="ExternalInput")
    covt_d = nc.dram_tensor("covt", (P, TT), F32, kind="ExternalInput")
    bqk_d = nc.dram_tensor("bqk", (P, NMT), F32, kind="ExternalInput")
    w2rep_d = nc.dram_tensor("w2rep", (P, H), BF16, kind="ExternalInput")
    bv8_d = nc.dram_tensor("bv8", (1, H), BF16, kind="ExternalInput")
    biasm8_d = nc.dram_tensor("biasm8", (1, H), BF16, kind="ExternalInput")
    mf8_d = nc.dram_tensor("mf8", (32, TT * 2 * P), F8, kind="ExternalInput")
    h8_d = nc.dram_tensor("h8", (4, P, 8 * H), F8, kind="ExternalInput")
    wqk8_d = nc.dram_tensor("wqk8", (P, KD * 2 * H), F8, kind="ExternalInput")
    wv8_d = nc.dram_tensor("wv8", (P, KD * H), F8, kind="ExternalInput")
    wm8_d = nc.dram_tensor("wm8", (P, KD * H), F8, kind="ExternalInput")
    whi8_d = nc.dram_tensor("whi8", (P, KD * H), F8, kind="ExternalInput")
    whid8_d = nc.dram_tensor("whid8", (P, KD * H), F8, kind="ExternalInput")
    wlod8_d = nc.dram_tensor("wlod8", (P, KD * H), F8, kind="ExternalInput")
    hthi_d = nc.dram_tensor("hthi", (8, P, 4 * KD * P), F8, kind="ExternalInput")
    htlo_d = nc.dram_tensor("htlo", (8, P, 4 * KD * P), F8, kind="ExternalInput")
    hbf_d = nc.dram_tensor("hbf", (8, P, 4 * H), BF16, kind="ExternalInput")
    out_d = nc.dram_tensor("out", (TT, P, H), BF16, kind="ExternalOutput")

    logit_scale = float((1.0 / np.sqrt(np.float32(DH))) / (AQK * AQK))
    z2p_scale = float(AW / (AV * AM))

    with tile.TileContext(nc) as tc:
        with tc.tile_pool(name="singles", bufs=1) as singles:
            mp8_sb = singles.tile([P, TT, S], F8)
            covt_sb = singles.tile([P, TT], F32)
            bqk_sb = singles.tile([P, NMT], F32)
            w2rep_sb = singles.tile([P, H], BF16)
            bv8_sb = singles.tile([1, H], BF16)
            biasm8_sb = singles.tile([1, H], BF16)
            mf8_sb = singles.tile([32, TT, 2, P], F8)
            wqk8_sb = singles.tile([P, KD, 2 * H], F8)
            wv8_sb = singles.tile([P, KD, H], F8)
            wm8_sb = singles.tile([P, KD, H], F8)
            whi8_sb = singles.tile([P, KD, H], F8)
            whid8_sb = singles.tile([P, KD, H], F8)
            wlod8_sb = singles.tile([P, KD, H], F8)
            hbf_sb = singles.tile([P, TT, H], BF16)
            sentT8 = singles.tile([P, KD, S], F8)
            qkT_bf = singles.tile([P, NMT, S], BF16)
            v_bf = singles.tile([S, H], BF16)
            ctxT8 = singles.tile([P, KD, S], F8)
            z2p8 = singles.tile([32, 2, H], F8)
            identbf = singles.tile([P, P], BF16)
            ones_bf = singles.tile([1, S], BF16)

            make_identity(nc, identbf)
            nc.vector.memset(ones_bf, 1.0)
            # small feeds first
            nc.sync.dma_start(mp8_sb, mp8_d.ap().rearrange("p (c s) -> p c s", c=TT))
            nc.sync.dma_start(covt_sb, covt_d.ap())
            nc.sync.dma_start(bqk_sb, bqk_d.ap())
            nc.sync.dma_start(w2rep_sb, w2rep_d.ap())
            nc.sync.dma_start(bv8_sb, bv8_d.ap())
            nc.sync.dma_start(biasm8_sb, biasm8_d.ap())
            nc.sync.dma_start(
                mf8_sb, mf8_d.ap().rearrange("p (c i u) -> p c i u", c=TT, i=2)
            )

            # ---------- phase A: fp8 DoubleRow segment pooling ----------
            with tc.tile_pool(name="pha", bufs=1) as pha:
                with tc.tile_pool(name="psA", bufs=1, space="PSUM") as psA:
                    psent = psA.tile([S, H], F32, tag="sent", bufs=1)
                    for g in range(4):
                        h8t = pha.tile([P, 8, H], F8, tag="h8", bufs=2)
                        nc.sync.dma_start(
                            h8t, h8_d[g].rearrange("p (a d) -> p a d", a=8)
                        )
                        if g == 0:
                            # queue attention weights right behind first pool
                            # chunks; z1 weight trio behind them
                            nc.sync.dma_start(
                                wqk8_sb,
                                wqk8_d.ap().rearrange("p (k m) -> p k m", k=KD),
                            )
                            nc.sync.dma_start(
                                wv8_sb, wv8_d.ap().rearrange("p (k m) -> p k m", k=KD)
                            )
                            nc.sync.dma_start(
                                wm8_sb, wm8_d.ap().rearrange("p (k m) -> p k m", k=KD)
                            )
                        for a2 in range(4):
                            cc = 8 * g + 2 * a2
                            for nb in range(2):
                                nc.tensor.matmul(
                                    psent[:, nb * 512:(nb + 1) * 512],
                                    mp8_sb[:, cc:cc + 2, :],
                                    h8t[:, 2 * a2:2 * a2 + 2, nb * 512:(nb + 1) * 512],
                                    start=(g == 0 and a2 == 0),
                                    stop=(g == 3 and a2 == 3),
                                    perf_mode=DR,
                                )
                    nc.sync.dma_start(
                        whi8_sb, whi8_d.ap().rearrange("p (k m) -> p k m", k=KD)
                    )
                    nc.sync.dma_start(
                        whid8_sb, whid8_d.ap().rearrange("p (k m) -> p k m", k=KD)
                    )
                    nc.sync.dma_start(
                        wlod8_sb, wlod8_d.ap().rearrange("p (k m) -> p k m", k=KD)
                    )
                    sent_bf = pha.tile([S, H], BF16, tag="sentbf", bufs=1)
                    nc.vector.tensor_copy(out=sent_bf, in_=psent)
                    for k in range(KD):
                        pst = psA.tile([P, S], BF16, tag="tr", bufs=2)
                        nc.tensor.transpose(
                            pst, sent_bf[:, k * P:(k + 1) * P], identbf[:S, :S]
                        )
                        nc.vector.tensor_copy(out=sentT8[:, k, :], in_=pst)

            # ---------- phase B: q|k, v, attention, z2' ----------
            with tc.tile_pool(name="phb", bufs=2) as phb:
                with tc.tile_pool(name="psB1", bufs=1, space="PSUM") as psB:
                    for mt in range(NMT):
                        psqk = psB.tile([P, S], F32, tag="qk", bufs=2)
                        for u in range(KP):
                            nc.tensor.matmul(
                                psqk,
                                wqk8_sb[:, 2 * u:2 * u + 2, mt * P:(mt + 1) * P],
                                sentT8[:, 2 * u:2 * u + 2, :],
                                start=(u == 0), stop=(u == KP - 1),
                                perf_mode=DR,
                            )
                        nc.vector.tensor_scalar(
                            out=qkT_bf[:, mt, :], in0=psqk,
                            scalar1=bqk_sb[:, mt:mt + 1], scalar2=None,
                            op0=mybir.AluOpType.add,
                        )

                    psv = psB.tile([S, H], F32, tag="v", bufs=1)
                    for u in range(KP):
                        for nb in range(2):
                            nc.tensor.matmul(
                                psv[:, nb * 512:(nb + 1) * 512],
                                sentT8[:, 2 * u:2 * u + 2, :],
                                wv8_sb[:, 2 * u:2 * u + 2, nb * 512:(nb + 1) * 512],
                                start=(u == 0), stop=False,
                                perf_mode=DR,
                            )
                    for nb in range(2):
                        nc.tensor.matmul(
                            psv[:, nb * 512:(nb + 1) * 512],
                            ones_bf, bv8_sb[:1, nb * 512:(nb + 1) * 512],
                            start=False, stop=True,
                        )
                    nc.vector.tensor_copy(out=v_bf, in_=psv)

                # attention heads (per-head softmax)
                with tc.tile_pool(name="psB2", bufs=1, space="PSUM") as psB:
                    for hh in range(NH):
                        po = (hh % 2) * 64
                        mt = hh // 2
                        qT_h = qkT_bf[po:po + 64, mt, :]
                        kT_h = qkT_bf[po:po + 64, NMT // 2 + mt, :]
                        ps_sc = psB.tile([S, S], F32, tag="sc", bufs=2)
                        nc.tensor.matmul(ps_sc, qT_h, kT_h, start=True, stop=True)
                        ex = phb.tile([S, S], F32, tag="ex")
                        nc.scalar.activation(
                            out=ex, in_=ps_sc,
                            func=mybir.ActivationFunctionType.Exp,
                            scale=logit_scale,
                        )
                        sm = phb.tile([S, 1], F32, tag="sm")
                        nc.vector.reduce_sum(out=sm, in_=ex, axis=mybir.AxisListType.X)
                        nc.vector.reciprocal(out=sm, in_=sm)
                        at = phb.tile([S, S], BF16, tag="at")
                        nc.vector.tensor_scalar_mul(out=at, in0=ex, scalar1=sm)
                        ps_t = psB.tile([S, S], BF16, tag="att", bufs=2)
                        nc.tensor.transpose(ps_t, at, identbf[:S, :S])
                        atT = phb.tile([S, S], BF16, tag="atT")
                        nc.vector.tensor_copy(out=atT, in_=ps_t)
                        ps_c = psB.tile([S, S], F32, tag="ctx", bufs=2)
                        nc.tensor.matmul(
                            ps_c, v_bf[:, hh * 64:(hh + 1) * 64], atT,
                            start=True, stop=True,
                        )
                        nc.vector.tensor_copy(out=ctxT8[po:po + 64, mt, :], in_=ps_c)

                # z2' halves -> z2p8 [32, 2, H] (AW-scaled fp8)
                with tc.tile_pool(name="psB3", bufs=1, space="PSUM") as psB:
                    for half in range(2):
                        psz2 = psB.tile([32, H], F32, tag="z2", bufs=2)
                        s0 = half * 32
                        for u in range(KP):
                            for nb in range(2):
                                nc.tensor.matmul(
                                    psz2[:, nb * 512:(nb + 1) * 512],
                                    ctxT8[:, 2 * u:2 * u + 2, s0:s0 + 32],
                                    wm8_sb[:, 2 * u:2 * u + 2, nb * 512:(nb + 1) * 512],
                                    start=(u == 0), stop=False,
                                    perf_mode=DR,
                                )
                        for nb in range(2):
                            nc.tensor.matmul(
                                psz2[:, nb * 512:(nb + 1) * 512],
                                ones_bf[:1, :32],
                                biasm8_sb[:1, nb * 512:(nb + 1) * 512],
                                start=False, stop=True,
                            )
                        nc.vector.tensor_scalar_mul(
                            out=z2p8[:, half, :], in0=psz2, scalar1=z2p_scale
                        )

            # ---------- phase C: scorer over token tiles ----------
            with tc.tile_pool(name="phc", bufs=2) as phc:
                with tc.tile_pool(name="psC", bufs=1, space="PSUM") as psC:
                    for q in range(8):
                        hthi_t = phc.tile([P, 4, KD, P], F8, tag="hi", bufs=2)
                        nc.sync.dma_start(
                            hthi_t,
                            hthi_d[q].rearrange("p (a k u) -> p a k u", a=4, k=KD),
                        )
                        htlo_t = phc.tile([P, 4, KD, P], F8, tag="lo", bufs=2)
                        nc.sync.dma_start(
                            htlo_t,
                            htlo_d[q].rearrange("p (a k u) -> p a k u", a=4, k=KD),
                        )
                        nc.sync.dma_start(
                            hbf_sb[:, 4 * q:4 * q + 4, :],
                            hbf_d[q].rearrange("p (a d) -> p a d", a=4),
                        )
                        for a in range(4):
                            c = 4 * q + a
                            psZ = psC.tile([P, H], F32, tag="z", bufs=3)
                            for u in range(KP):
                                sl = slice(2 * u, 2 * u + 2)
                                for nb in range(2):
                                    nbs = slice(nb * 512, (nb + 1) * 512)
                                    nc.tensor.matmul(
                                        psZ[:, nbs], hthi_t[:, a, sl, :],
                                        whi8_sb[:, sl, nbs],
                                        start=(u == 0), stop=False, perf_mode=DR,
                                    )
                                for nb in range(2):
                                    nbs = slice(nb * 512, (nb + 1) * 512)
                                    nc.tensor.matmul(
                                        psZ[:, nbs], hthi_t[:, a, sl, :],
                                        wlod8_sb[:, sl, nbs],
                                        start=False, stop=False, perf_mode=DR,
                                    )
                                for nb in range(2):
                                    nbs = slice(nb * 512, (nb + 1) * 512)
                                    nc.tensor.matmul(
                                        psZ[:, nbs], htlo_t[:, a, sl, :],
                                        whid8_sb[:, sl, nbs],
                                        start=False, stop=False, perf_mode=DR,
                                    )
                            for nb in range(2):
                                nbs = slice(nb * 512, (nb + 1) * 512)
                                nc.tensor.matmul(
                                    psZ[:, nbs], mf8_sb[:, c, :, :],
                                    z2p8[:, :, nbs],
                                    start=False, stop=(nb == 1), perf_mode=DR,
                                )
                            scratch = phc.tile([P, H], BF16, tag="scr", bufs=2)
                            acc = phc.tile([P, 1], F32, tag="acc", bufs=2)
                            nc.vector.scalar_tensor_tensor(
                                out=scratch, in0=psZ, scalar=0.0, in1=w2rep_sb,
                                op0=mybir.AluOpType.max, op1=mybir.AluOpType.mult,
                                accum_out=acc,
                            )
                            sc = phc.tile([P, 1], F32, tag="scl", bufs=2)
                            nc.vector.scalar_tensor_tensor(
                                out=sc, in0=acc, scalar=float(b2val),
                                in1=covt_sb[:, c:c + 1],
                                op0=mybir.AluOpType.add, op1=mybir.AluOpType.mult,
                            )
                            outt = phc.tile([P, H], BF16, tag="out", bufs=4)
                            nc.vector.scalar_tensor_tensor(
                                out=outt, in0=hbf_sb[:, c, :], scalar=sc,
                                in1=hbf_sb[:, c, :],
                                op0=mybir.AluOpType.mult, op1=mybir.AluOpType.add,
                            )
                            nc.scalar.dma_start(out_d[c], outt)
    _split_multi_waits(nc)
    return nc


def _preprocess(context_hidden, sentence_boundaries, in_proj_w, in_proj_b,
                out_w, out_b, w1, b1, w2, b2):
    """Host-side index preprocessing + parameter folding (shared across cores)."""
    starts = np.asarray(sentence_boundaries)[:, :, 0].astype(np.int64)   # [B,S]
    ends = np.asarray(sentence_boundaries)[:, :, 1].astype(np.int64)     # [B,S]
    t = np.arange(T, dtype=np.int64)
    member = (t[None, :, None] >= starts[:, None, :]) & (
        t[None, :, None] < ends[:, None, :]
    )                                                        # [B,T,S]
    mf = member.astype(np.float32)
    counts = np.clip(mf.sum(axis=1), 1.0, None)              # [B,S]
    mpool = mf / counts[:, None, :]                          # [B,T,S]
    sid = np.argmax(member, axis=2)                          # [B,T]
    covered = member.any(axis=2)                             # [B,T]
    memberF = np.eye(S, dtype=np.float32)[sid] * covered[..., None].astype(np.float32)

    in_proj_w = np.asarray(in_proj_w, np.float32)
    in_proj_b = np.asarray(in_proj_b, np.float32)
    w1 = np.asarray(w1, np.float32)
    w2 = np.asarray(w2, np.float32)

    def pack_dmaj(Wt, scale, quant=True):
        # W [m, d] -> [P, KD, m-len] fp8 with [p, k, m] = W[m, k*128+p]
        arr = (Wt * scale).astype(F8NP).astype(np.float32) if quant else Wt * scale
        a = np.ascontiguousarray(
            arr.T.reshape(KD, P, -1).transpose(1, 0, 2)
        )
        return a

    wqk = in_proj_w[:2 * H]                                  # [2H, H]
    wqk8 = pack_dmaj(wqk, AQK).astype(F8NP).reshape(P, KD * 2 * H)
    bqk = np.ascontiguousarray(
        (AQK * in_proj_b[:2 * H]).reshape(NMT, P).T
    ).astype(np.float32)

    # v: wv8[p, k, d'] = f8(AV * in_proj_w[2H + d', k*128+p])
    wv = in_proj_w[2 * H:]                                   # [d', d]
    wv8 = np.ascontiguousarray(
        (AV * wv).astype(F8NP).astype(np.float32)
        .T.reshape(KD, P, H).transpose(1, 0, 2)
    ).astype(F8NP).reshape(P, KD * H)
    bv8 = (AV * in_proj_b[2 * H:])[None, :].astype(BF)

    w1a = w1[:, :H]
    W1b = w1[:, H:]
    Wm = W1b @ np.asarray(out_w, np.float32)                 # [j, d']
    biasm = W1b @ np.asarray(out_b, np.float32) + np.asarray(b1, np.float32)

    whi_f = (AW * w1a).astype(F8NP).astype(np.float32)       # [j, d]
    wlo_f = ((AW * w1a - whi_f) * RK).astype(F8NP).astype(np.float32)
    whid_f = (whi_f / RK)
    wlod_f = (wlo_f / RK)

    def pack_w(Wjd):
        return np.ascontiguousarray(
            Wjd.T.reshape(KD, P, H).transpose(1, 0, 2)
        ).astype(F8NP).reshape(P, KD * H)

    whi8 = pack_w(whi_f)
    whid8 = pack_w(whid_f)
    wlod8 = pack_w(wlod_f)
    wm8 = np.ascontiguousarray(
        (AM * Wm).astype(F8NP).astype(np.float32)
        .T.reshape(KD, P, H).transpose(1, 0, 2)
    ).astype(F8NP).reshape(P, KD * H)
    biasm8 = (AV * AM * biasm)[None, :].astype(BF)
    w2rep = np.ascontiguousarray(
        np.broadcast_to((w2[0] / AW).astype(BF), (P, H))
    )
    b2val = float(np.asarray(b2).reshape(-1)[0])

    shared = dict(
        wqk8=wqk8, bqk=bqk, wv8=wv8, bv8=bv8, wm8=wm8,
        whi8=whi8, whid8=whid8, wlod8=wlod8, biasm8=biasm8, w2rep=w2rep,
    )

    in_maps = []
    for b in range(B):
        hb = np.ascontiguousarray(np.asarray(context_hidden)[b]).astype(np.float32)
        hi_f = hb.astype(F8NP).astype(np.float32)
        lo_f = ((hb - hi_f) * RK).astype(F8NP)

        h8 = np.ascontiguousarray(
            hi_f.astype(F8NP).reshape(4, 8, P, H).transpose(0, 2, 1, 3)
        ).reshape(4, P, 8 * H)
        hbf = np.ascontiguousarray(
            hb.astype(BF).reshape(8, 4, P, H).transpose(0, 2, 1, 3)
        ).reshape(8, P, 4 * H)

        def pack_ht(x8):
            # [q, p, a, k, u] = x8[(4q+a)*128+u, k*128+p]
            a = x8.reshape(8, 4, P, KD, P).transpose(0, 4, 1, 3, 2)
            return np.ascontiguousarray(a).reshape(8, P, 4 * KD * P)

        hthi = pack_ht(hi_f.astype(F8NP))
        htlo = pack_ht(lo_f)

        mp8 = np.ascontiguousarray(
            mpool[b].astype(F8NP).reshape(TT, P, S).transpose(1, 0, 2)
        ).reshape(P, TT * S)
        mf8 = np.ascontiguousarray(
            memberF[b].astype(F8NP).reshape(TT, P, 2, 32).transpose(3, 0, 2, 1)
        ).reshape(32, TT * 2 * P)
        covt = np.ascontiguousarray(
            covered[b].astype(np.float32).reshape(TT, P).T
        )
        in_maps.append(dict(
            shared, h8=h8, hbf=hbf, hthi=hthi, htlo=htlo,
            mp8=mp8, mf8=mf8, covt=covt,
        ))
    return in_maps, b2val


def kernel(**inputs) -> np.ndarray:
    in_maps, b2val = _preprocess(**inputs)
    key = ("nc", b2val)
    if key not in _CACHE:
        _CACHE[key] = _build(b2val)
    nc = _CACHE[key]
    res = run_bass_kernel_spmd(nc, in_maps, core_ids=list(range(B)))
    out = np.stack(
        [res.results[b]["out"].reshape(T, H) for b in range(B)], axis=0
    )
    return out.astype(np.float32)
